# revision 50
# baseline (speedup 1.0000x reference)
"""Trainium2 Bass kernel for a 2-layer GCN (nn_EvenLamerGCN).

reference semantics (PyG GCNConv x2, eval mode):
    deg[i]  = 1 + indeg(i)                (self-loops added)
    dinv    = deg ** -0.5
    h  = relu(A_hat @ (x @ W1) + b1),  A_hat = D^-1/2 (A + I) D^-1/2
    o  = A_hat @ (h @ W2) + b2
    return o, log_softmax(o, axis=1)

Distribution: nodes sharded over 8 NeuronCores (12500/core, padded to
12544), edges partitioned by destination core.  The per-edge norm is
folded into per-node row scalings:
    out = dinv * ( sum_{e: dst=i} T[src_e] + T[i] + sqrt(deg_i)*b ),
    T = dinv * (x @ W)

Gather tables are laid out in 4 "super-sections": section s holds ALL
cores' local row-quarter s contiguously, so one sub-AllGather per
section completes one int16-addressable gather window.  Per layer:
  1. dense matmul per section -> sub-AllGather, pipelined into the
     gather stream (AG_s instructions are injected between gather
     pieces once their inputs are known-ready, keeping the in-order
     Pool queue from stalling)
  2. per-edge dma_gather of T[src] rows (128-row chunks, 16-chunk
     pieces) round-robined over all 4 SWDGE queues (the Q7 descriptor
     generator is the machine bottleneck at ~8ns/row on one queue,
     ~2.5ns/row across four)
  3. segment-sum via one-hot matmul on PE.  Each dst block's PSUM
     chain is seeded with an identity-matmul of the self-loop row and
     a rank-1 (sqrt(deg) x bias) matmul, so DVE does nothing but the
     one-hot builds and 3 drain-adds per block; all scaling/relu/cast
     runs on the Scalar engine (which reads PSUM directly)
  4. when a dst block's last chunk drains (during the final gather
     window), the next phase's per-block work runs immediately (t2
     construction for layer 1, output + log_softmax for layer 2)
x is supplied pre-transposed ([din, nloc]) so the x @ W1 phase needs
no on-device transposes.  Edges are laid out per (dst-block,
src-window) cell with a uniform chunk quota so the instruction stream
is identical on all 8 cores (SPMD, one NEFF); all per-core variation
lives in input data.
"""

import sys

for _p in ("/opt/trn_rl_repo", "/root/.axon_site/_ro/trn_rl_repo"):
    if _p not in sys.path:
        sys.path.insert(0, _p)

from contextlib import ExitStack
from dataclasses import dataclass

import numpy as np

import concourse.bass as bass
import concourse.mybir as mybir
import concourse.tile as tile
from concourse import bacc
from concourse.bass import ds, ts
from concourse.bass_utils import run_bass_kernel_spmd
from concourse.masks import make_identity

F32 = mybir.dt.float32
BF16 = mybir.dt.bfloat16
I16 = mybir.dt.int16
AF = mybir.ActivationFunctionType
ALU = mybir.AluOpType


@dataclass(frozen=True)
class Cfg:
    n: int = 100000          # nodes
    din: int = 512           # input features
    dh: int = 128            # hidden features
    dout: int = 40           # output features
    cores: int = 8
    max_piece: int = 16      # chunks per gather instruction
    nsec: int = 4            # table super-sections (= gather windows)
    gbufs: int = 12          # gather tile ring depth

    @property
    def nsh(self):           # real nodes per core
        return self.n // self.cores

    @property
    def nloc(self):          # padded nodes per core (multiple of 128)
        return ((self.nsh + 127) // 128) * 128

    @property
    def nt(self):            # 128-node dst blocks per core
        return self.nloc // 128

    @property
    def stb(self):           # table blocks per section (uniform)
        # one above the minimum: the last section then covers fewer real
        # rows, dropping its window's chunk quota by one
        base = -(-self.nt // self.nsec)
        if (base + 1) * 128 * self.cores <= 32768:   # int16 window limit
            return base + 1
        return base

    @property
    def sec_rows(self):      # local table rows per section (uniform)
        return self.stb * 128

    @property
    def tloc(self):          # local table rows (padded to nsec sections)
        return self.nsec * self.sec_rows

    @property
    def wsize(self):         # gather window = all cores' section s
        return self.sec_rows * self.cores

    @property
    def wbases(self):
        return tuple(s * self.wsize for s in range(self.nsec))

    @property
    def trows(self):         # rows in the gathered tables
        return self.cores * self.tloc

    @property
    def dh2(self):           # layer-2 useful width
        return max(64, ((self.dout + 63) // 64) * 64)

    @property
    def dt2(self):           # layer-2 table row width (256B rows)
        return max(128, self.dh2)

    @property
    def kt(self):            # k-tiles in the first matmul
        return self.din // 128

    @property
    def nwin(self):
        return self.nsec


@dataclass(frozen=True)
class Plan:
    quotas: tuple          # chunks per (window) cell, per dst block
    sections: tuple        # per window: list of piece sizes (in chunks)

    @property
    def total_chunks(self):
        return sum(sum(s) for s in self.sections)


# ----------------------------------------------------------------------------
# CPU-side preprocessing
# ----------------------------------------------------------------------------

def preprocess(cfg: Cfg, edge_index: np.ndarray):
    c = cfg
    src = np.asarray(edge_index[0], dtype=np.int64)
    dst = np.asarray(edge_index[1], dtype=np.int64)

    deg = np.bincount(dst, minlength=c.n).astype(np.float32) + 1.0
    dinv_pt = np.ones((c.cores, 128, c.nt), np.float32)
    sqdeg_r = np.ones((c.cores, 1, c.nloc), np.float32)
    for ci in range(c.cores):
        dl = np.ones(c.nloc, np.float32)
        dl[: c.nsh] = deg[ci * c.nsh : (ci + 1) * c.nsh]
        dinv_pt[ci] = (1.0 / np.sqrt(dl)).reshape(c.nt, 128).T
        sqdeg_r[ci, 0] = np.sqrt(dl)

    wbases = np.array(c.wbases)

    # table row of source node i: uniform sections of local rows, table
    # laid out [section][core][local row within section]
    core_s = src // c.nsh
    lr = src - core_s * c.nsh
    s_all = np.minimum(lr // c.sec_rows, c.nsec - 1)
    r_all = wbases[s_all] + core_s * c.sec_rows + (lr - s_all * c.sec_rows)
    w_all = s_all

    core_all = dst // c.nsh
    dloc_all = dst - core_all * c.nsh
    b_all = dloc_all // 128
    id_all = dloc_all % 128

    # count edges per (core, block, window) -> uniform chunk quotas
    cell_key = (core_all * c.nt + b_all) * c.nwin + w_all
    counts = np.bincount(cell_key, minlength=c.cores * c.nt * c.nwin)
    counts = counts.reshape(c.cores, c.nt, c.nwin)
    quotas = tuple(int(-(-counts[:, :, w].max() // 128)) for w in range(c.nwin))

    # piece sizes (chunks) per window section
    sections = []
    for w in range(c.nwin):
        sec = c.nt * quotas[w]
        sizes = []
        while sec > 0:
            sizes.append(min(c.max_piece, sec))
            sec -= sizes[-1]
        sections.append(tuple(sizes))
    plan = Plan(quotas=quotas, sections=tuple(sections))

    total_chunks = plan.total_chunks
    slots = total_chunks * 128

    idx16 = np.zeros((c.cores, 128, slots // 16), np.int16)
    ids_f32 = np.empty((c.cores, 128, total_chunks), np.float32)

    order = np.lexsort((r_all, w_all, b_all, core_all))
    so_r, so_w, so_b, so_core, so_id = (
        r_all[order], w_all[order], b_all[order], core_all[order], id_all[order]
    )
    core_starts = np.searchsorted(so_core, np.arange(c.cores + 1))

    for ci in range(c.cores):
        lo, hi = core_starts[ci], core_starts[ci + 1]
        rr, ii = so_r[lo:hi], so_id[lo:hi]
        rel = np.zeros(slots, np.int64)      # window-relative gather rows
        ids = np.full(slots, -1.0, np.float32)
        # slot offset of window section w
        sec_off = np.cumsum([0] + [c.nt * q * 128 for q in quotas])
        pos = 0
        # sorted order within a core is (b, w, r); cells land at
        # sec_off[w] + b * quotas[w] * 128
        for b in range(c.nt):
            for w in range(c.nwin):
                cnt = counts[ci, b, w]
                if cnt:
                    off = sec_off[w] + b * quotas[w] * 128
                    rel[off : off + cnt] = rr[pos : pos + cnt] - wbases[w]
                    ids[off : off + cnt] = ii[pos : pos + cnt]
                    pos += cnt
        assert pos == hi - lo
        assert rel.min() >= 0
        for w in range(c.nwin):
            seg = rel[sec_off[w] : sec_off[w + 1]]
            assert seg.max(initial=0) < c.wsize

        v = rel.reshape(-1, 16)              # slot i at [i%16, i//16]
        wrapped = np.ascontiguousarray(v.T)  # [16, slots/16]
        idx16[ci] = np.tile(wrapped, (8, 1)).astype(np.int16)
        ids_f32[ci] = ids.reshape(total_chunks, 128).T

    return dinv_pt, sqdeg_r, idx16, ids_f32, plan


# ----------------------------------------------------------------------------
# Device kernel
# ----------------------------------------------------------------------------

def build(nc, tc, cfg: Cfg, plan: Plan):
    c = cfg
    RG = [list(range(c.cores))]
    total_chunks = plan.total_chunks
    slots = total_chunks * 128
    nsec = c.nsec
    stb = c.stb                             # table blocks per section

    x_t = nc.dram_tensor("x_t", [c.din, c.nloc], BF16, kind="ExternalInput").ap()
    w1 = nc.dram_tensor("w1", [c.din, c.dh], BF16, kind="ExternalInput").ap()
    w2 = nc.dram_tensor("w2", [c.dh, c.dt2], F32, kind="ExternalInput").ap()
    b1r = nc.dram_tensor("b1r", [1, c.dh], F32, kind="ExternalInput").ap()
    b2r = nc.dram_tensor("b2r", [1, c.dt2], F32, kind="ExternalInput").ap()
    dinvp = nc.dram_tensor("dinvp", [128, c.nt], F32, kind="ExternalInput").ap()
    sqdegr = nc.dram_tensor("sqdegr", [1, c.nloc], F32, kind="ExternalInput").ap()
    idx16 = nc.dram_tensor("idx16", [128, slots // 16], I16, kind="ExternalInput").ap()
    idsf = nc.dram_tensor("idsf", [128, total_chunks], BF16, kind="ExternalInput").ap()
    out_h = nc.dram_tensor("out_h", [c.nloc, c.dh2], F32, kind="ExternalOutput").ap()
    out_ls = nc.dram_tensor("out_ls", [c.nloc, c.dh2], F32, kind="ExternalOutput").ap()

    # local table shards (uniform sections) + full (shared) gather tables
    t1_loc = nc.dram_tensor("t1_loc", [c.tloc, c.dh], BF16, kind="Internal").ap()
    t1_full = nc.dram_tensor(
        "t1_full", [c.trows, c.dh], BF16, kind="Internal", addr_space="Shared"
    ).ap()
    t2_loc = nc.dram_tensor("t2_loc", [c.tloc, c.dt2], BF16, kind="Internal").ap()
    t2_cm = nc.dram_tensor(
        "t2_cm", [c.trows, c.dt2], BF16, kind="Internal", addr_space="Shared"
    ).ap()
    t2_full = nc.dram_tensor(
        "t2_full", [c.trows, c.dt2], BF16, kind="Internal"
    ).ap()

    with ExitStack() as st:
        cpool = st.enter_context(tc.tile_pool(name="consts", bufs=1))
        accp = st.enter_context(tc.tile_pool(name="acc", bufs=1))
        gp = st.enter_context(tc.tile_pool(name="gp", bufs=c.gbufs))
        pp = st.enter_context(tc.tile_pool(name="pp", bufs=6))
        ppsum = st.enter_context(tc.tile_pool(name="ppsum", bufs=5, space="PSUM"))
        p0 = st.enter_context(tc.tile_pool(name="p0", bufs=3))
        p0ps = st.enter_context(tc.tile_pool(name="p0ps", bufs=2, space="PSUM"))
        p0psT = st.enter_context(tc.tile_pool(name="p0psT", bufs=1, space="PSUM"))

        # ---- constants ----
        ident = cpool.tile([128, 128], F32)
        make_identity(nc, ident)
        identb = cpool.tile([128, 128], BF16)
        make_identity(nc, identb)
        w1sb = cpool.tile([128, c.kt, c.dh], BF16)
        nc.sync.dma_start(w1sb, w1.rearrange("(o p) f -> p o f", p=128))
        w2sb = cpool.tile([128, c.dt2], F32)
        nc.sync.dma_start(w2sb, w2)
        b1sb = cpool.tile([1, c.dh], F32)
        nc.sync.dma_start(b1sb, b1r)
        b2sb = cpool.tile([1, c.dt2], F32)
        nc.sync.dma_start(b2sb, b2r)
        dinv = cpool.tile([128, c.nt], F32)
        nc.sync.dma_start(dinv, dinvp)
        iota = cpool.tile([128, c.max_piece, 128], BF16)
        nc.gpsimd.iota(iota, pattern=[[0, c.max_piece], [1, 128]], base=0,
                       channel_multiplier=0,
                       allow_small_or_imprecise_dtypes=True)

        # ---- whole-run gather indices + one-hot ids, loaded once ----
        sit = cpool.tile([128, total_chunks * 8], I16)
        nc.sync.dma_start(sit, idx16)
        sid = cpool.tile([128, total_chunks], BF16)
        nc.sync.dma_start(sid, idsf)

        # ---- phase 0: T1 = dinv * (x @ W1) ----
        xtv = x_t.rearrange("(k p) n -> p k n", p=128)   # [128, kt, nloc]

        def phase0_block(t):
            xTt = p0.tile([128, c.kt, 128], BF16, tag="xTt")
            nc.sync.dma_start(xTt, xtv[:, :, ts(t, 128)])
            hps = p0ps.tile([128, c.dh], F32, tag="hps")
            for j in range(c.kt):
                nc.tensor.matmul(
                    hps, lhsT=xTt[:, j, :], rhs=w1sb[:, j, :],
                    start=(j == 0), stop=(j == c.kt - 1),
                )
            hsb = p0.tile([128, c.dh], BF16, tag="hsb")
            nc.scalar.activation(hsb, hps, AF.Copy, scale=dinv[:, t : t + 1])
            nc.sync.dma_start(t1_loc[ts(t, 128), :], hsb)

        # Sub-AllGather of one section s: local rows [s*sr, (s+1)*sr) of
        # every core concatenate into gather window s.
        def emit_ag1(s):
            nc.gpsimd.collective_compute(
                "AllGather", ALU.bypass, replica_groups=RG,
                ins=[t1_loc[ds(s * c.sec_rows, c.sec_rows), :].opt()],
                outs=[t1_full[ds(c.wbases[s], c.wsize), :].opt()],
            )

        for t in range(c.nt):
            phase0_block(t)
            if t == stb - 1:     # section 0 complete -> unblock window 0
                emit_ag1(0)

        # ---- edge aggregation ----
        qctr = [0]
        LOOK = 2                # gather/stt emission lookahead (pieces)
        HLAG = 5                # hook emission lag (pieces)

        def edge_phase(table_full, t_loc, acc, d, dt, brow, hook, inject, worder):
            """acc[:, b, :d] = sum_edges T[src] + T[self] + sqdeg*b  per block.

            Software-pipelined emission: gather+one-hot of piece k+LOOK are
            emitted before the matmuls of piece k (so DVE one-hot builds
            never queue behind PSUM drains), and hook bodies for blocks
            completed in piece k are emitted after piece k+1's matmuls (so
            their PE/scalar round-trips never stall the piece stream).
            inject: {gather_piece_index: fn} slots collectives into the
            Pool instruction stream between gathers.
            """
            # precompute piece descriptors, windows in processing order
            wchunk0, acc_ch = [], 0
            for sizes in plan.sections:
                wchunk0.append(acc_ch)
                acc_ch += sum(sizes)
            pieces = []          # (w, nch, loc)
            for w in worder:
                loc = 0
                for nch in plan.sections[w]:
                    pieces.append((w, nch, loc))
                    loc += nch
            npc = len(pieces)
            wfirst, wlast = worder[0], worder[-1]
            gtile = {}
            sttile = {}

            def emit_fetch(k):
                w, nch, loc = pieces[k]
                if k in inject:
                    inject[k]()
                ch0 = wchunk0[w] + loc
                g = gp.tile([128, c.max_piece, dt], BF16, tag="gt")
                nc.gpsimd.dma_gather(
                    g[:, :nch, :], table_full[ds(c.wbases[w], c.wsize), :],
                    sit[:, ch0 * 8 : (ch0 + nch) * 8],
                    num_idxs=nch * 128, num_idxs_reg=nch * 128, elem_size=dt,
                    single_packet=False, queue_num=qctr[0] % 4,
                )
                qctr[0] += 1
                stt = pp.tile([128, c.max_piece, 128], BF16, tag="stt")
                nc.vector.tensor_tensor(
                    stt[:, :nch, :], iota[:, :nch, :],
                    sid[:, wchunk0[w] + loc : wchunk0[w] + loc + nch, None]
                    .to_broadcast((128, nch, 128)),
                    ALU.is_equal,
                )
                gtile[k] = g
                sttile[k] = stt

            tv = t_loc.rearrange("(b p) f -> p b f", p=128)

            def seed(b):
                sd = p0.tile([128, dt], BF16, tag="sd")
                nc.sync.dma_start(sd, tv[:, b, :])
                sq = p0.tile([1, 128], F32, tag="sq")
                nc.sync.dma_start(sq, sqdegr[:, ts(b, 128)])
                return sd, sq

            state = {"b": 0, "k_in_block": 0, "ps": None}

            def emit_consume(k, newhooks):
                w, nch, loc = pieces[k]
                q = plan.quotas[w]
                first, last = w == wfirst, w == wlast
                if loc == 0:
                    state["b"] = 0
                    state["k_in_block"] = 0
                g, stt = gtile.pop(k), sttile.pop(k)
                for j in range(nch):
                    b = state["b"]
                    if state["k_in_block"] == 0:
                        ps_new = ppsum.tile([128, d], F32, tag="ps")
                        state["ps"] = ps_new
                        if first:
                            # chain seeds: self-loop row + rank-1 bias
                            sd, sq = seed(b)
                            nc.tensor.matmul(
                                state["ps"], lhsT=identb, rhs=sd[:, :d],
                                start=True, stop=False,
                            )
                            nc.tensor.matmul(
                                state["ps"], lhsT=sq, rhs=brow[:, :d],
                                start=False, stop=False,
                            )
                    nc.tensor.matmul(
                        state["ps"], lhsT=stt[:, j, :], rhs=g[:, j, :d],
                        start=(state["k_in_block"] == 0 and not first),
                        stop=(state["k_in_block"] == q - 1),
                    )
                    state["k_in_block"] += 1
                    if state["k_in_block"] == q:
                        ps = state["ps"]
                        if first:
                            nc.vector.tensor_copy(acc[:, b, :], ps)
                        else:
                            nc.vector.tensor_tensor(
                                acc[:, b, :], acc[:, b, :], ps, ALU.add
                            )
                            if last:
                                newhooks.append(b)
                        state["b"] += 1
                        state["k_in_block"] = 0

            for k in range(LOOK):
                emit_fetch(k)
            pending = []         # (piece, [blocks]) hook backlog
            for k in range(npc):
                if k + LOOK < npc:
                    emit_fetch(k + LOOK)
                newhooks = []
                emit_consume(k, newhooks)
                if newhooks:
                    pending.append((k, newhooks))
                while pending and pending[0][0] <= k - HLAG:
                    for b in pending.pop(0)[1]:
                        hook(b)
            for _, hs in pending:
                for b in hs:
                    hook(b)
            assert state["b"] == c.nt and state["k_in_block"] == 0

        # ---- layer 1, with per-block t2 construction + sub-AllGathers ----
        acc1 = accp.tile([128, c.nt, c.dh], BF16)

        def hook1(b):
            # g1 = dinv * relu(dinv * agg + b1); t2 rows = g1 @ W2
            zb = acc1[:, b, :]
            zr = p0.tile([128, c.dh], F32, tag="zr")
            nc.scalar.activation(zr, zb, AF.Relu, scale=dinv[:, b : b + 1])
            zs = p0.tile([128, c.dh], F32, tag="zs")
            nc.scalar.activation(zs, zr, AF.Copy, scale=dinv[:, b : b + 1])
            tps = p0psT.tile([128, 128], F32, tag="tps2")
            nc.tensor.transpose(tps, zs, ident)
            gT = p0.tile([128, 128], F32, tag="gT")
            nc.scalar.activation(gT, tps, AF.Copy)
            h2ps = p0ps.tile([128, c.dt2], F32, tag="hps")
            nc.tensor.matmul(h2ps, lhsT=gT, rhs=w2sb, start=True, stop=True)
            h2sb = p0.tile([128, c.dt2], BF16, tag="h2sb")
            nc.scalar.activation(h2sb, h2ps, AF.Copy)
            nc.sync.dma_start(t2_loc[ts(b, 128), :], h2sb)

        edge_phase(
            t1_full, t1_loc, acc1, c.dh, c.dh, b1sb, hook1,
            inject={10: lambda: emit_ag1(1),
                    15: lambda: emit_ag1(2),
                    20: lambda: emit_ag1(3)},
            worder=(0, 1, 2, 3),
        )

        # single merged AllGather for the whole t2 table (contiguous
        # core-major output), then 4 local repack DMAs produce the
        # section-major gather windows; window w's gathers wait only on
        # repack w via normal range tracking.
        nc.gpsimd.collective_compute(
            "AllGather", ALU.bypass, replica_groups=RG,
            ins=[t2_loc.opt()], outs=[t2_cm.opt()],
        )
        t2cmv = t2_cm.rearrange("(cc s r) f -> s cc r f", cc=c.cores, s=nsec)
        for w in range(nsec):
            nc.sync.dma_start(
                t2_full[ds(c.wbases[w], c.wsize), :].rearrange(
                    "(cc r) f -> cc r f", cc=c.cores
                ),
                t2cmv[w],
            )

        # ---- layer 2, per-block scale/max/exp; all DMAs batched at end ----
        acc2 = accp.tile([128, c.nt, c.dh2], F32)
        se_all = cpool.tile([128, c.nt], F32)
        znall = cpool.tile([128, c.nt, c.dout], F32)
        ohv = out_h.rearrange("(t p) f -> p t f", p=128)
        olv = out_ls.rearrange("(t p) f -> p t f", p=128)

        def hook2(b):
            # in place: acc2_b <- dinv * acc2_b  (the final h rows)
            nc.scalar.activation(acc2[:, b, :], acc2[:, b, :], AF.Copy,
                                 scale=dinv[:, b : b + 1])
            mx = p0.tile([128, 1], F32, tag="mx")
            nc.vector.tensor_reduce(mx, acc2[:, b, : c.dout],
                                    mybir.AxisListType.X, ALU.max)
            nc.vector.tensor_tensor(
                znall[:, b, :], acc2[:, b, : c.dout],
                mx.to_broadcast((128, c.dout)), ALU.subtract,
            )
            e1 = p0.tile([128, c.dout], F32, tag="e1")
            nc.scalar.activation(e1, znall[:, b, :], AF.Exp)
            nc.vector.tensor_reduce(se_all[:, b : b + 1], e1,
                                    mybir.AxisListType.X, ALU.add)

        edge_phase(t2_full, t2_loc, acc2, c.dh2, c.dt2, b2sb, hook2,
                   inject={}, worder=(0, 1, 2, 3))

        # batched tail: h out, then out_ls = (z - mx) - ln(sum exp)
        nc.sync.dma_start(ohv, acc2)
        lnall = cpool.tile([128, c.nt], F32)
        nc.scalar.activation(lnall, se_all, AF.Ln)
        nc.vector.tensor_tensor(
            znall, znall,
            lnall[:, :, None].to_broadcast((128, c.nt, c.dout)), ALU.subtract,
        )
        nc.sync.dma_start(olv[:, :, : c.dout], znall)


# ----------------------------------------------------------------------------
# Host entry point
# ----------------------------------------------------------------------------

_CACHE = {}


def _get_compiled(cfg: Cfg, plan: Plan):
    key = (cfg, plan)
    if key not in _CACHE:
        nc = bacc.Bacc(
            "TRN2", target_bir_lowering=False, debug=False,
            num_devices=cfg.cores, num_swdge_queues=4,
            dynamic_dma_scratch_size=32768,
        )
        with tile.TileContext(nc) as tc:
            build(nc, tc, cfg, plan)
        nc.compile()
        _CACHE[key] = nc
    return _CACHE[key]


def make_in_maps(cfg: Cfg, x, W1, b1, W2, b2, dinv_pt, sqdeg_r, idx16, ids_f32):
    import ml_dtypes

    c = cfg
    x = np.asarray(x, np.float32)
    w2p = np.zeros((c.dh, c.dt2), np.float32)
    w2p[:, : c.dout] = np.asarray(W2, np.float32)
    b1rep = np.asarray(b1, np.float32)[None, :]
    b2p = np.zeros((1, c.dt2), np.float32)
    b2p[0, : c.dout] = np.asarray(b2, np.float32)
    w1c = np.ascontiguousarray(
        np.asarray(W1, np.float32).astype(ml_dtypes.bfloat16)
    )

    in_maps = []
    for ci in range(c.cores):
        xs = np.zeros((c.din, c.nloc), ml_dtypes.bfloat16)
        xs[:, : c.nsh] = np.ascontiguousarray(
            x[ci * c.nsh : (ci + 1) * c.nsh].T
        ).astype(ml_dtypes.bfloat16)
        in_maps.append({
            "x_t": xs,
            "w1": w1c,
            "w2": w2p,
            "b1r": b1rep,
            "b2r": b2p,
            "dinvp": np.ascontiguousarray(dinv_pt[ci]),
            "sqdegr": np.ascontiguousarray(sqdeg_r[ci]),
            "idx16": np.ascontiguousarray(idx16[ci]),
            "idsf": np.ascontiguousarray(ids_f32[ci].astype(ml_dtypes.bfloat16)),
        })
    return in_maps


def _ensure_ntff_hook():
    """Install the axon NTFF profile hook if the image's antenv lacks it."""
    import types

    try:
        from antenv.axon_hooks import get_axon_ntff_profile_hook  # noqa: F401
        return
    except ImportError:
        pass
    import antenv

    m = types.ModuleType("antenv.axon_hooks")
    m._hook = None
    m.set_axon_ntff_profile_hook = lambda h: setattr(m, "_hook", h)
    m.get_axon_ntff_profile_hook = lambda: m._hook
    sys.modules["antenv.axon_hooks"] = m
    antenv.axon_hooks = m
    try:
        from trn_agent_boot.trn_boot import _ntff_profile_via_ctypes

        h = _ntff_profile_via_ctypes("/opt/axon/libaxon_pjrt.so")
        if h is not None:
            m._hook = h
    except Exception as e:
        print(f"ntff hook install failed: {e}")

    from concourse import bass_utils as bu

    bu.upload_artifacts = lambda tmpdir: tmpdir


def run(cfg: Cfg, inputs: dict, trace: bool = False):
    if trace:
        _ensure_ntff_hook()
    dinv_pt, sqdeg_r, idx16, ids_f32, plan = preprocess(cfg, inputs["edge_index"])
    nc = _get_compiled(cfg, plan)
    in_maps = make_in_maps(
        cfg, inputs["x"], inputs["W1"], inputs["b1"], inputs["W2"], inputs["b2"],
        dinv_pt, sqdeg_r, idx16, ids_f32,
    )
    res = run_bass_kernel_spmd(
        nc, in_maps, core_ids=list(range(cfg.cores)), trace=trace
    )
    c = cfg
    h = np.concatenate(
        [res.results[ci]["out_h"][: c.nsh, : c.dout] for ci in range(c.cores)], axis=0
    )
    ls = np.concatenate(
        [res.results[ci]["out_ls"][: c.nsh, : c.dout] for ci in range(c.cores)], axis=0
    )
    return (h, ls), res


def kernel(**inputs):
    (h, ls), _ = run(Cfg(), inputs)
    return h, ls


# revision 58
# speedup vs baseline: 1.0942x; 1.0942x over previous
"""Trainium2 Bass kernel for a 2-layer GCN (nn_EvenLamerGCN).

reference semantics (PyG GCNConv x2, eval mode):
    deg[i]  = 1 + indeg(i)                (self-loops added)
    dinv    = deg ** -0.5
    h  = relu(A_hat @ (x @ W1) + b1),  A_hat = D^-1/2 (A + I) D^-1/2
    o  = A_hat @ (h @ W2) + b2
    return o, log_softmax(o, axis=1)

Distribution: nodes sharded over 8 NeuronCores (12500/core, padded to
12544), edges partitioned by destination core.  The per-edge norm is
folded into per-node row scalings:
    out = dinv * ( sum_{e: dst=i} T[src_e] + T[i] + sqrt(deg_i)*b ),
    T = dinv * (x @ W)

Gather tables are laid out in 4 "super-sections": section s holds ALL
cores' local row-quarter s contiguously, so one sub-AllGather per
section completes one int16-addressable gather window.  Per layer:
  1. dense matmul per section -> sub-AllGather, pipelined into the
     gather stream (AG_s instructions are injected between gather
     pieces once their inputs are known-ready, keeping the in-order
     Pool queue from stalling)
  2. per-edge dma_gather of T[src] rows (128-row chunks, 16-chunk
     pieces) round-robined over all 4 SWDGE queues (the Q7 descriptor
     generator is the machine bottleneck at ~8ns/row on one queue,
     ~2.5ns/row across four)
  3. segment-sum via one-hot matmul on PE.  Each dst block's PSUM
     chain is seeded with an identity-matmul of the self-loop row and
     a rank-1 (sqrt(deg) x bias) matmul, so DVE does nothing but the
     one-hot builds and 3 drain-adds per block; all scaling/relu/cast
     runs on the Scalar engine (which reads PSUM directly)
  4. when a dst block's last chunk drains (during the final gather
     window), the next phase's per-block work runs immediately (t2
     construction for layer 1, output + log_softmax for layer 2)
x is supplied pre-transposed ([din, nloc]) so the x @ W1 phase needs
no on-device transposes.  Edges are laid out per (dst-block,
src-window) cell with a uniform chunk quota so the instruction stream
is identical on all 8 cores (SPMD, one NEFF); all per-core variation
lives in input data.
"""

import sys

for _p in ("/opt/trn_rl_repo", "/root/.axon_site/_ro/trn_rl_repo"):
    if _p not in sys.path:
        sys.path.insert(0, _p)

from contextlib import ExitStack
from dataclasses import dataclass

import numpy as np

import concourse.bass as bass
import concourse.mybir as mybir
import concourse.tile as tile
from concourse import bacc
from concourse.bass import ds, ts
from concourse.bass_utils import run_bass_kernel_spmd
from concourse.masks import make_identity

F32 = mybir.dt.float32
BF16 = mybir.dt.bfloat16
I16 = mybir.dt.int16
AF = mybir.ActivationFunctionType
ALU = mybir.AluOpType


@dataclass(frozen=True)
class Cfg:
    n: int = 100000          # nodes
    din: int = 512           # input features
    dh: int = 128            # hidden features
    dout: int = 40           # output features
    cores: int = 8
    max_piece: int = 16      # chunks per gather instruction
    nsec: int = 4            # table super-sections (= gather windows)
    gbufs: int = 12          # gather tile ring depth

    @property
    def nsh(self):           # real nodes per core
        return self.n // self.cores

    @property
    def nloc(self):          # padded nodes per core (multiple of 128)
        return ((self.nsh + 127) // 128) * 128

    @property
    def nt(self):            # 128-node dst blocks per core
        return self.nloc // 128

    @property
    def stb(self):           # table blocks per section (uniform)
        # one above the minimum: the last section then covers fewer real
        # rows, dropping its window's chunk quota by one
        base = -(-self.nt // self.nsec)
        if (base + 1) * 128 * self.cores <= 32768:   # int16 window limit
            return base + 1
        return base

    @property
    def sec_rows(self):      # local table rows per section (uniform)
        return self.stb * 128

    @property
    def tloc(self):          # local table rows (padded to nsec sections)
        return self.nsec * self.sec_rows

    @property
    def wsize(self):         # gather window = all cores' section s
        return self.sec_rows * self.cores

    @property
    def wbases(self):
        return tuple(s * self.wsize for s in range(self.nsec))

    @property
    def trows(self):         # rows in the gathered tables
        return self.cores * self.tloc

    @property
    def dh2(self):           # layer-2 useful width
        return max(64, ((self.dout + 63) // 64) * 64)

    @property
    def dt2(self):           # layer-2 table row width (256B rows)
        return max(128, self.dh2)

    @property
    def kt(self):            # k-tiles in the first matmul
        return self.din // 128

    @property
    def nwin(self):
        return self.nsec


@dataclass(frozen=True)
class Plan:
    quotas: tuple          # chunks per (window) cell, per dst block
    sections: tuple        # per window: list of piece sizes (in chunks)

    @property
    def total_chunks(self):
        return sum(sum(s) for s in self.sections)


# ----------------------------------------------------------------------------
# CPU-side preprocessing
# ----------------------------------------------------------------------------

def _rotseq(cfg: Cfg):
    """Block order for the last window: last-section blocks first, so the
    t2 sub-AllGather layer 2 consumes first is also the first to fire."""
    lo3 = (cfg.nsec - 1) * cfg.stb
    return list(range(lo3, cfg.nt)) + list(range(lo3))


def _rotpos(cfg: Cfg):
    pos = [0] * cfg.nt
    for p, b in enumerate(_rotseq(cfg)):
        pos[b] = p
    return pos


def preprocess(cfg: Cfg, edge_index: np.ndarray):
    c = cfg
    src = np.asarray(edge_index[0], dtype=np.int64)
    dst = np.asarray(edge_index[1], dtype=np.int64)

    deg = np.bincount(dst, minlength=c.n).astype(np.float32) + 1.0
    dinv_pt = np.ones((c.cores, 128, c.nt), np.float32)
    sqdeg_r = np.ones((c.cores, 1, c.nloc), np.float32)
    for ci in range(c.cores):
        dl = np.ones(c.nloc, np.float32)
        dl[: c.nsh] = deg[ci * c.nsh : (ci + 1) * c.nsh]
        dinv_pt[ci] = (1.0 / np.sqrt(dl)).reshape(c.nt, 128).T
        sqdeg_r[ci, 0] = np.sqrt(dl)

    wbases = np.array(c.wbases)

    # table row of source node i: uniform sections of local rows, table
    # laid out [section][core][local row within section]
    core_s = src // c.nsh
    lr = src - core_s * c.nsh
    s_all = np.minimum(lr // c.sec_rows, c.nsec - 1)
    r_all = wbases[s_all] + core_s * c.sec_rows + (lr - s_all * c.sec_rows)
    w_all = s_all

    core_all = dst // c.nsh
    dloc_all = dst - core_all * c.nsh
    b_all = dloc_all // 128
    id_all = dloc_all % 128

    # count edges per (core, block, window) -> uniform chunk quotas
    cell_key = (core_all * c.nt + b_all) * c.nwin + w_all
    counts = np.bincount(cell_key, minlength=c.cores * c.nt * c.nwin)
    counts = counts.reshape(c.cores, c.nt, c.nwin)
    quotas = tuple(int(-(-counts[:, :, w].max() // 128)) for w in range(c.nwin))

    # piece sizes (chunks) per window section
    sections = []
    for w in range(c.nwin):
        sec = c.nt * quotas[w]
        sizes = []
        while sec > 0:
            sizes.append(min(c.max_piece, sec))
            sec -= sizes[-1]
        sections.append(tuple(sizes))
    plan = Plan(quotas=quotas, sections=tuple(sections))

    total_chunks = plan.total_chunks
    slots = total_chunks * 128

    idx16 = np.zeros((c.cores, 128, slots // 16), np.int16)
    ids_f32 = np.empty((c.cores, 128, total_chunks), np.float32)

    order = np.lexsort((r_all, w_all, b_all, core_all))
    so_r, so_w, so_b, so_core, so_id = (
        r_all[order], w_all[order], b_all[order], core_all[order], id_all[order]
    )
    core_starts = np.searchsorted(so_core, np.arange(c.cores + 1))

    for ci in range(c.cores):
        lo, hi = core_starts[ci], core_starts[ci + 1]
        rr, ii = so_r[lo:hi], so_id[lo:hi]
        rel = np.zeros(slots, np.int64)      # window-relative gather rows
        ids = np.full(slots, -1.0, np.float32)
        # slot offset of window section w
        sec_off = np.cumsum([0] + [c.nt * q * 128 for q in quotas])
        pos = 0
        # sorted order within a core is (b, w, r); cells land at
        # sec_off[w] + pos_in_window(b) * quotas[w] * 128 (last window's
        # cells rotated to match the rotated hook/processing order)
        rotpos = _rotpos(c)
        for b in range(c.nt):
            for w in range(c.nwin):
                cnt = counts[ci, b, w]
                if cnt:
                    p = rotpos[b] if w == c.nwin - 1 else b
                    off = sec_off[w] + p * quotas[w] * 128
                    rel[off : off + cnt] = rr[pos : pos + cnt] - wbases[w]
                    ids[off : off + cnt] = ii[pos : pos + cnt]
                    pos += cnt
        assert pos == hi - lo
        assert rel.min() >= 0
        for w in range(c.nwin):
            seg = rel[sec_off[w] : sec_off[w + 1]]
            assert seg.max(initial=0) < c.wsize

        v = rel.reshape(-1, 16)              # slot i at [i%16, i//16]
        wrapped = np.ascontiguousarray(v.T)  # [16, slots/16]
        idx16[ci] = np.tile(wrapped, (8, 1)).astype(np.int16)
        ids_f32[ci] = ids.reshape(total_chunks, 128).T

    return dinv_pt, sqdeg_r, idx16, ids_f32, plan


# ----------------------------------------------------------------------------
# Device kernel
# ----------------------------------------------------------------------------

def build(nc, tc, cfg: Cfg, plan: Plan):
    c = cfg
    RG = [list(range(c.cores))]
    total_chunks = plan.total_chunks
    slots = total_chunks * 128
    nsec = c.nsec
    stb = c.stb                             # table blocks per section

    x_t = nc.dram_tensor("x_t", [c.din, c.nloc], BF16, kind="ExternalInput").ap()
    w1 = nc.dram_tensor("w1", [c.din, c.dh], BF16, kind="ExternalInput").ap()
    w2 = nc.dram_tensor("w2", [c.dh, c.dt2], F32, kind="ExternalInput").ap()
    b1r = nc.dram_tensor("b1r", [1, c.dh], F32, kind="ExternalInput").ap()
    b2r = nc.dram_tensor("b2r", [1, c.dt2], F32, kind="ExternalInput").ap()
    dinvp = nc.dram_tensor("dinvp", [128, c.nt], F32, kind="ExternalInput").ap()
    sqdegr = nc.dram_tensor("sqdegr", [1, c.nloc], F32, kind="ExternalInput").ap()
    idx16 = nc.dram_tensor("idx16", [128, slots // 16], I16, kind="ExternalInput").ap()
    idsf = nc.dram_tensor("idsf", [128, total_chunks], BF16, kind="ExternalInput").ap()
    out_h = nc.dram_tensor("out_h", [c.nloc, c.dh2], F32, kind="ExternalOutput").ap()
    out_ls = nc.dram_tensor("out_ls", [c.nloc, c.dh2], F32, kind="ExternalOutput").ap()

    # local table shards (uniform sections) + full (shared) gather tables
    t1_loc = nc.dram_tensor("t1_loc", [c.tloc, c.dh], BF16, kind="Internal").ap()
    t1_full = nc.dram_tensor(
        "t1_full", [c.trows, c.dh], BF16, kind="Internal", addr_space="Shared"
    ).ap()
    t2_loc = nc.dram_tensor("t2_loc", [c.tloc, c.dt2], BF16, kind="Internal").ap()
    t2_full = nc.dram_tensor(
        "t2_full", [c.trows, c.dt2], BF16, kind="Internal", addr_space="Shared"
    ).ap()

    with ExitStack() as st:
        cpool = st.enter_context(tc.tile_pool(name="consts", bufs=1))
        accp = st.enter_context(tc.tile_pool(name="acc", bufs=1))
        gp = st.enter_context(tc.tile_pool(name="gp", bufs=c.gbufs))
        pp = st.enter_context(tc.tile_pool(name="pp", bufs=6))
        ppsum = st.enter_context(tc.tile_pool(name="ppsum", bufs=5, space="PSUM"))
        p0 = st.enter_context(tc.tile_pool(name="p0", bufs=3))
        p0ps = st.enter_context(tc.tile_pool(name="p0ps", bufs=2, space="PSUM"))
        p0psT = st.enter_context(tc.tile_pool(name="p0psT", bufs=1, space="PSUM"))

        # ---- constants ----
        ident = cpool.tile([128, 128], F32)
        make_identity(nc, ident)
        identb = cpool.tile([128, 128], BF16)
        make_identity(nc, identb)
        w1sb = cpool.tile([128, c.kt, c.dh], BF16)
        nc.sync.dma_start(w1sb, w1.rearrange("(o p) f -> p o f", p=128))
        w2sb = cpool.tile([128, c.dt2], F32)
        nc.sync.dma_start(w2sb, w2)
        b1sb = cpool.tile([1, c.dh], F32)
        nc.sync.dma_start(b1sb, b1r)
        b2sb = cpool.tile([1, c.dt2], F32)
        nc.sync.dma_start(b2sb, b2r)
        dinv = cpool.tile([128, c.nt], F32)
        nc.sync.dma_start(dinv, dinvp)
        iota = cpool.tile([128, c.max_piece, 128], BF16)
        nc.gpsimd.iota(iota, pattern=[[0, c.max_piece], [1, 128]], base=0,
                       channel_multiplier=0,
                       allow_small_or_imprecise_dtypes=True)

        # ---- whole-run gather indices + one-hot ids, loaded once ----
        sit = cpool.tile([128, total_chunks * 8], I16)
        nc.sync.dma_start(sit, idx16)
        sid = cpool.tile([128, total_chunks], BF16)
        nc.sync.dma_start(sid, idsf)

        # ---- phase 0: T1 = dinv * (x @ W1) ----
        xtv = x_t.rearrange("(k p) n -> p k n", p=128)   # [128, kt, nloc]

        def phase0_block(t):
            xTt = p0.tile([128, c.kt, 128], BF16, tag="xTt")
            nc.sync.dma_start(xTt, xtv[:, :, ts(t, 128)])
            hps = p0ps.tile([128, c.dh], F32, tag="hps")
            for j in range(c.kt):
                nc.tensor.matmul(
                    hps, lhsT=xTt[:, j, :], rhs=w1sb[:, j, :],
                    start=(j == 0), stop=(j == c.kt - 1),
                )
            hsb = p0.tile([128, c.dh], BF16, tag="hsb")
            nc.scalar.activation(hsb, hps, AF.Copy, scale=dinv[:, t : t + 1])
            nc.sync.dma_start(t1_loc[ts(t, 128), :], hsb)

        # Sub-AllGather of one section s: local rows [s*sr, (s+1)*sr) of
        # every core concatenate into gather window s.
        def emit_ag1(s):
            nc.gpsimd.collective_compute(
                "AllGather", ALU.bypass, replica_groups=RG,
                ins=[t1_loc[ds(s * c.sec_rows, c.sec_rows), :].opt()],
                outs=[t1_full[ds(c.wbases[s], c.wsize), :].opt()],
            )

        for t in range(c.nt):
            phase0_block(t)
            if t == stb - 1:     # section 0 complete -> unblock window 0
                emit_ag1(0)

        # ---- edge aggregation ----
        qctr = [0]
        LOOK = 2                # gather/stt emission lookahead (pieces)
        HLAG = 5                # hook emission lag (pieces)

        def edge_phase(table_full, t_loc, acc, d, dt, brow, hook, inject, worder):
            """acc[:, b, :d] = sum_edges T[src] + T[self] + sqdeg*b  per block.

            Software-pipelined emission: gather+one-hot of piece k+LOOK are
            emitted before the matmuls of piece k (so DVE one-hot builds
            never queue behind PSUM drains), and hook bodies for blocks
            completed in piece k are emitted after piece k+1's matmuls (so
            their PE/scalar round-trips never stall the piece stream).
            inject: {gather_piece_index: fn} slots collectives into the
            Pool instruction stream between gathers.
            """
            # precompute piece descriptors, windows in processing order
            wchunk0, acc_ch = [], 0
            for sizes in plan.sections:
                wchunk0.append(acc_ch)
                acc_ch += sum(sizes)
            pieces = []          # (w, nch, loc)
            for w in worder:
                loc = 0
                for nch in plan.sections[w]:
                    pieces.append((w, nch, loc))
                    loc += nch
            npc = len(pieces)
            wfirst, wlast = worder[0], worder[-1]
            rotseq = _rotseq(c)

            def blk_at(w, pos):
                return rotseq[pos] if w == c.nwin - 1 else pos
            gtile = {}
            sttile = {}

            def emit_fetch(k):
                w, nch, loc = pieces[k]
                if k in inject:
                    inject[k]()
                ch0 = wchunk0[w] + loc
                g = gp.tile([128, c.max_piece, dt], BF16, tag="gt")
                nc.gpsimd.dma_gather(
                    g[:, :nch, :], table_full[ds(c.wbases[w], c.wsize), :],
                    sit[:, ch0 * 8 : (ch0 + nch) * 8],
                    num_idxs=nch * 128, num_idxs_reg=nch * 128, elem_size=dt,
                    single_packet=False, queue_num=qctr[0] % 4,
                )
                qctr[0] += 1
                stt = pp.tile([128, c.max_piece, 128], BF16, tag="stt")
                nc.vector.tensor_tensor(
                    stt[:, :nch, :], iota[:, :nch, :],
                    sid[:, wchunk0[w] + loc : wchunk0[w] + loc + nch, None]
                    .to_broadcast((128, nch, 128)),
                    ALU.is_equal,
                )
                gtile[k] = g
                sttile[k] = stt

            tv = t_loc.rearrange("(b p) f -> p b f", p=128)

            def seed(b):
                sd = p0.tile([128, dt], BF16, tag="sd")
                nc.sync.dma_start(sd, tv[:, b, :])
                sq = p0.tile([1, 128], F32, tag="sq")
                nc.sync.dma_start(sq, sqdegr[:, ts(b, 128)])
                return sd, sq

            state = {"b": 0, "k_in_block": 0, "ps": None}

            def emit_consume(k, newhooks):
                w, nch, loc = pieces[k]
                q = plan.quotas[w]
                first, last = w == wfirst, w == wlast
                if loc == 0:
                    state["b"] = 0
                    state["k_in_block"] = 0
                g, stt = gtile.pop(k), sttile.pop(k)
                for j in range(nch):
                    b = blk_at(w, state["b"])
                    if state["k_in_block"] == 0:
                        ps_new = ppsum.tile([128, d], F32, tag="ps")
                        state["ps"] = ps_new
                        if first:
                            # chain seeds: self-loop row + rank-1 bias
                            sd, sq = seed(b)
                            nc.tensor.matmul(
                                state["ps"], lhsT=identb, rhs=sd[:, :d],
                                start=True, stop=False,
                            )
                            nc.tensor.matmul(
                                state["ps"], lhsT=sq, rhs=brow[:, :d],
                                start=False, stop=False,
                            )
                    nc.tensor.matmul(
                        state["ps"], lhsT=stt[:, j, :], rhs=g[:, j, :d],
                        start=(state["k_in_block"] == 0 and not first),
                        stop=(state["k_in_block"] == q - 1),
                    )
                    state["k_in_block"] += 1
                    if state["k_in_block"] == q:
                        ps = state["ps"]
                        if first:
                            nc.vector.tensor_copy(acc[:, b, :], ps)
                        else:
                            nc.vector.tensor_tensor(
                                acc[:, b, :], acc[:, b, :], ps, ALU.add
                            )
                            if last:
                                newhooks.append(b)
                        state["b"] += 1
                        state["k_in_block"] = 0

            for k in range(LOOK):
                emit_fetch(k)
            pending = []         # (piece, [blocks]) hook backlog
            for k in range(npc):
                if k + LOOK < npc:
                    emit_fetch(k + LOOK)
                newhooks = []
                emit_consume(k, newhooks)
                if newhooks:
                    pending.append((k, newhooks))
                while pending and pending[0][0] <= k - HLAG:
                    for b in pending.pop(0)[1]:
                        hook(b)
            for _, hs in pending:
                for b in hs:
                    hook(b)
            assert state["b"] == c.nt and state["k_in_block"] == 0

        # ---- layer 1, with per-block t2 construction + sub-AllGathers ----
        acc1 = accp.tile([128, c.nt, c.dh], BF16)

        def hook1(b):
            # g1 = dinv * relu(dinv * agg + b1); t2 rows = g1 @ W2
            zb = acc1[:, b, :]
            zr = p0.tile([128, c.dh], F32, tag="zr")
            nc.scalar.activation(zr, zb, AF.Relu, scale=dinv[:, b : b + 1])
            zs = p0.tile([128, c.dh], F32, tag="zs")
            nc.scalar.activation(zs, zr, AF.Copy, scale=dinv[:, b : b + 1])
            tps = p0psT.tile([128, 128], F32, tag="tps2")
            nc.tensor.transpose(tps, zs, ident)
            gT = p0.tile([128, 128], F32, tag="gT")
            nc.scalar.activation(gT, tps, AF.Copy)
            h2ps = p0ps.tile([128, c.dt2], F32, tag="hps")
            nc.tensor.matmul(h2ps, lhsT=gT, rhs=w2sb, start=True, stop=True)
            h2sb = p0.tile([128, c.dt2], BF16, tag="h2sb")
            nc.scalar.activation(h2sb, h2ps, AF.Copy)
            nc.sync.dma_start(t2_loc[ts(b, 128), :], h2sb)
            # hooks run in rotated order: each table section finishes
            # contiguously, and its sub-AllGather fires immediately
            s = b // stb
            if b == min((s + 1) * stb, c.nt) - 1:
                nc.gpsimd.collective_compute(
                    "AllGather", ALU.bypass, replica_groups=RG,
                    ins=[t2_loc[ds(s * c.sec_rows, c.sec_rows), :].opt()],
                    outs=[t2_full[ds(c.wbases[s], c.wsize), :].opt()],
                )

        edge_phase(
            t1_full, t1_loc, acc1, c.dh, c.dh, b1sb, hook1,
            inject={10: lambda: emit_ag1(1),
                    20: lambda: emit_ag1(2),
                    30: lambda: emit_ag1(3)},
            worder=(0, 1, 2, 3),
        )



        # ---- layer 2, per-block scale/max/exp; all DMAs batched at end ----
        acc2 = accp.tile([128, c.nt, c.dh2], F32)
        se_all = cpool.tile([128, c.nt], F32)
        znall = cpool.tile([128, c.nt, c.dout], F32)
        ohv = out_h.rearrange("(t p) f -> p t f", p=128)
        olv = out_ls.rearrange("(t p) f -> p t f", p=128)

        def hook2(b):
            # in place: acc2_b <- dinv * acc2_b  (the final h rows)
            nc.scalar.activation(acc2[:, b, :], acc2[:, b, :], AF.Copy,
                                 scale=dinv[:, b : b + 1])
            mx = p0.tile([128, 1], F32, tag="mx")
            nc.vector.tensor_reduce(mx, acc2[:, b, : c.dout],
                                    mybir.AxisListType.X, ALU.max)
            nc.vector.tensor_tensor(
                znall[:, b, :], acc2[:, b, : c.dout],
                mx.to_broadcast((128, c.dout)), ALU.subtract,
            )
            e1 = p0.tile([128, c.dout], F32, tag="e1")
            nc.scalar.activation(e1, znall[:, b, :], AF.Exp)
            nc.vector.tensor_reduce(se_all[:, b : b + 1], e1,
                                    mybir.AxisListType.X, ALU.add)

        edge_phase(t2_full, t2_loc, acc2, c.dh2, c.dt2, b2sb, hook2,
                   inject={}, worder=(3, 0, 1, 2))

        # batched tail: h out, then out_ls = (z - mx) - ln(sum exp)
        nc.sync.dma_start(ohv, acc2)
        lnall = cpool.tile([128, c.nt], F32)
        nc.scalar.activation(lnall, se_all, AF.Ln)
        nc.vector.tensor_tensor(
            znall, znall,
            lnall[:, :, None].to_broadcast((128, c.nt, c.dout)), ALU.subtract,
        )
        nc.sync.dma_start(olv[:, :, : c.dout], znall)


# ----------------------------------------------------------------------------
# Host entry point
# ----------------------------------------------------------------------------

_CACHE = {}


def _get_compiled(cfg: Cfg, plan: Plan):
    key = (cfg, plan)
    if key not in _CACHE:
        nc = bacc.Bacc(
            "TRN2", target_bir_lowering=False, debug=False,
            num_devices=cfg.cores, num_swdge_queues=4,
            dynamic_dma_scratch_size=32768,
        )
        with tile.TileContext(nc) as tc:
            build(nc, tc, cfg, plan)
        nc.compile()
        _CACHE[key] = nc
    return _CACHE[key]


def make_in_maps(cfg: Cfg, x, W1, b1, W2, b2, dinv_pt, sqdeg_r, idx16, ids_f32):
    import ml_dtypes

    c = cfg
    x = np.asarray(x, np.float32)
    w2p = np.zeros((c.dh, c.dt2), np.float32)
    w2p[:, : c.dout] = np.asarray(W2, np.float32)
    b1rep = np.asarray(b1, np.float32)[None, :]
    b2p = np.zeros((1, c.dt2), np.float32)
    b2p[0, : c.dout] = np.asarray(b2, np.float32)
    w1c = np.ascontiguousarray(
        np.asarray(W1, np.float32).astype(ml_dtypes.bfloat16)
    )

    in_maps = []
    for ci in range(c.cores):
        xs = np.zeros((c.din, c.nloc), ml_dtypes.bfloat16)
        xs[:, : c.nsh] = np.ascontiguousarray(
            x[ci * c.nsh : (ci + 1) * c.nsh].T
        ).astype(ml_dtypes.bfloat16)
        in_maps.append({
            "x_t": xs,
            "w1": w1c,
            "w2": w2p,
            "b1r": b1rep,
            "b2r": b2p,
            "dinvp": np.ascontiguousarray(dinv_pt[ci]),
            "sqdegr": np.ascontiguousarray(sqdeg_r[ci]),
            "idx16": np.ascontiguousarray(idx16[ci]),
            "idsf": np.ascontiguousarray(ids_f32[ci].astype(ml_dtypes.bfloat16)),
        })
    return in_maps


def _ensure_ntff_hook():
    """Install the axon NTFF profile hook if the image's antenv lacks it."""
    import types

    try:
        from antenv.axon_hooks import get_axon_ntff_profile_hook  # noqa: F401
        return
    except ImportError:
        pass
    import antenv

    m = types.ModuleType("antenv.axon_hooks")
    m._hook = None
    m.set_axon_ntff_profile_hook = lambda h: setattr(m, "_hook", h)
    m.get_axon_ntff_profile_hook = lambda: m._hook
    sys.modules["antenv.axon_hooks"] = m
    antenv.axon_hooks = m
    try:
        from trn_agent_boot.trn_boot import _ntff_profile_via_ctypes

        h = _ntff_profile_via_ctypes("/opt/axon/libaxon_pjrt.so")
        if h is not None:
            m._hook = h
    except Exception as e:
        print(f"ntff hook install failed: {e}")

    from concourse import bass_utils as bu

    bu.upload_artifacts = lambda tmpdir: tmpdir


def run(cfg: Cfg, inputs: dict, trace: bool = False):
    if trace:
        _ensure_ntff_hook()
    dinv_pt, sqdeg_r, idx16, ids_f32, plan = preprocess(cfg, inputs["edge_index"])
    nc = _get_compiled(cfg, plan)
    in_maps = make_in_maps(
        cfg, inputs["x"], inputs["W1"], inputs["b1"], inputs["W2"], inputs["b2"],
        dinv_pt, sqdeg_r, idx16, ids_f32,
    )
    res = run_bass_kernel_spmd(
        nc, in_maps, core_ids=list(range(cfg.cores)), trace=trace
    )
    c = cfg
    h = np.concatenate(
        [res.results[ci]["out_h"][: c.nsh, : c.dout] for ci in range(c.cores)], axis=0
    )
    ls = np.concatenate(
        [res.results[ci]["out_ls"][: c.nsh, : c.dout] for ci in range(c.cores)], axis=0
    )
    return (h, ls), res


def kernel(**inputs):
    (h, ls), _ = run(Cfg(), inputs)
    return h, ls


# revision 59
# speedup vs baseline: 1.0994x; 1.0048x over previous
"""Trainium2 Bass kernel for a 2-layer GCN (nn_EvenLamerGCN).

reference semantics (PyG GCNConv x2, eval mode):
    deg[i]  = 1 + indeg(i)                (self-loops added)
    dinv    = deg ** -0.5
    h  = relu(A_hat @ (x @ W1) + b1),  A_hat = D^-1/2 (A + I) D^-1/2
    o  = A_hat @ (h @ W2) + b2
    return o, log_softmax(o, axis=1)

Distribution: nodes sharded over 8 NeuronCores (12500/core, padded to
12544), edges partitioned by destination core.  The per-edge norm is
folded into per-node row scalings:
    out = dinv * ( sum_{e: dst=i} T[src_e] + T[i] + sqrt(deg_i)*b ),
    T = dinv * (x @ W)

Gather tables are laid out in 4 "super-sections": section s holds ALL
cores' local row-quarter s contiguously, so one sub-AllGather per
section completes one int16-addressable gather window.  Per layer:
  1. dense matmul per section -> sub-AllGather, pipelined into the
     gather stream (AG_s instructions are injected between gather
     pieces once their inputs are known-ready, keeping the in-order
     Pool queue from stalling)
  2. per-edge dma_gather of T[src] rows (128-row chunks, 16-chunk
     pieces) round-robined over all 4 SWDGE queues (the Q7 descriptor
     generator is the machine bottleneck at ~8ns/row on one queue,
     ~2.5ns/row across four)
  3. segment-sum via one-hot matmul on PE.  Each dst block's PSUM
     chain is seeded with an identity-matmul of the self-loop row and
     a rank-1 (sqrt(deg) x bias) matmul, so DVE does nothing but the
     one-hot builds and 3 drain-adds per block; all scaling/relu/cast
     runs on the Scalar engine (which reads PSUM directly)
  4. when a dst block's last chunk drains (during the final gather
     window), the next phase's per-block work runs immediately (t2
     construction for layer 1, output + log_softmax for layer 2)
x is supplied pre-transposed ([din, nloc]) so the x @ W1 phase needs
no on-device transposes.  Edges are laid out per (dst-block,
src-window) cell with a uniform chunk quota so the instruction stream
is identical on all 8 cores (SPMD, one NEFF); all per-core variation
lives in input data.
"""

import sys

for _p in ("/opt/trn_rl_repo", "/root/.axon_site/_ro/trn_rl_repo"):
    if _p not in sys.path:
        sys.path.insert(0, _p)

from contextlib import ExitStack
from dataclasses import dataclass

import numpy as np

import concourse.bass as bass
import concourse.mybir as mybir
import concourse.tile as tile
from concourse import bacc
from concourse.bass import ds, ts
from concourse.bass_utils import run_bass_kernel_spmd
from concourse.masks import make_identity

F32 = mybir.dt.float32
BF16 = mybir.dt.bfloat16
I16 = mybir.dt.int16
AF = mybir.ActivationFunctionType
ALU = mybir.AluOpType


@dataclass(frozen=True)
class Cfg:
    n: int = 100000          # nodes
    din: int = 512           # input features
    dh: int = 128            # hidden features
    dout: int = 40           # output features
    cores: int = 8
    max_piece: int = 16      # chunks per gather instruction
    nsec: int = 4            # table super-sections (= gather windows)
    gbufs: int = 12          # gather tile ring depth

    @property
    def nsh(self):           # real nodes per core
        return self.n // self.cores

    @property
    def nloc(self):          # padded nodes per core (multiple of 128)
        return ((self.nsh + 127) // 128) * 128

    @property
    def nt(self):            # 128-node dst blocks per core
        return self.nloc // 128

    @property
    def stb(self):           # table blocks per section (uniform)
        # one above the minimum: the last section then covers fewer real
        # rows, dropping its window's chunk quota by one
        base = -(-self.nt // self.nsec)
        if (base + 1) * 128 * self.cores <= 32768:   # int16 window limit
            return base + 1
        return base

    @property
    def sec_rows(self):      # local table rows per section (uniform)
        return self.stb * 128

    @property
    def tloc(self):          # local table rows (padded to nsec sections)
        return self.nsec * self.sec_rows

    @property
    def wsize(self):         # gather window = all cores' section s
        return self.sec_rows * self.cores

    @property
    def wbases(self):
        return tuple(s * self.wsize for s in range(self.nsec))

    @property
    def trows(self):         # rows in the gathered tables
        return self.cores * self.tloc

    @property
    def dh2(self):           # layer-2 useful width
        return max(64, ((self.dout + 63) // 64) * 64)

    @property
    def dt2(self):           # layer-2 table row width (256B rows)
        return max(128, self.dh2)

    @property
    def kt(self):            # k-tiles in the first matmul
        return self.din // 128

    @property
    def nwin(self):
        return self.nsec


@dataclass(frozen=True)
class Plan:
    quotas: tuple          # chunks per (window) cell, per dst block
    sections: tuple        # per window: list of piece sizes (in chunks)

    @property
    def total_chunks(self):
        return sum(sum(s) for s in self.sections)


# ----------------------------------------------------------------------------
# CPU-side preprocessing
# ----------------------------------------------------------------------------

def _rotseq(cfg: Cfg):
    """Block order for the last window: last-section blocks first, so the
    t2 sub-AllGather layer 2 consumes first is also the first to fire."""
    lo3 = (cfg.nsec - 1) * cfg.stb
    return list(range(lo3, cfg.nt)) + list(range(lo3))


def _rotpos(cfg: Cfg):
    pos = [0] * cfg.nt
    for p, b in enumerate(_rotseq(cfg)):
        pos[b] = p
    return pos


def preprocess(cfg: Cfg, edge_index: np.ndarray):
    c = cfg
    src = np.asarray(edge_index[0], dtype=np.int64)
    dst = np.asarray(edge_index[1], dtype=np.int64)

    deg = np.bincount(dst, minlength=c.n).astype(np.float32) + 1.0
    dinv_pt = np.ones((c.cores, 128, c.nt), np.float32)
    sqdeg_r = np.ones((c.cores, 1, c.nloc), np.float32)
    for ci in range(c.cores):
        dl = np.ones(c.nloc, np.float32)
        dl[: c.nsh] = deg[ci * c.nsh : (ci + 1) * c.nsh]
        dinv_pt[ci] = (1.0 / np.sqrt(dl)).reshape(c.nt, 128).T
        sqdeg_r[ci, 0] = np.sqrt(dl)

    wbases = np.array(c.wbases)

    # table row of source node i: uniform sections of local rows, table
    # laid out [section][core][local row within section]
    core_s = src // c.nsh
    lr = src - core_s * c.nsh
    s_all = np.minimum(lr // c.sec_rows, c.nsec - 1)
    r_all = wbases[s_all] + core_s * c.sec_rows + (lr - s_all * c.sec_rows)
    w_all = s_all

    core_all = dst // c.nsh
    dloc_all = dst - core_all * c.nsh
    b_all = dloc_all // 128
    id_all = dloc_all % 128

    # count edges per (core, block, window) -> uniform chunk quotas
    cell_key = (core_all * c.nt + b_all) * c.nwin + w_all
    counts = np.bincount(cell_key, minlength=c.cores * c.nt * c.nwin)
    counts = counts.reshape(c.cores, c.nt, c.nwin)
    quotas = tuple(int(-(-counts[:, :, w].max() // 128)) for w in range(c.nwin))

    # piece sizes (chunks) per window section
    sections = []
    for w in range(c.nwin):
        sec = c.nt * quotas[w]
        sizes = []
        while sec > 0:
            sizes.append(min(c.max_piece, sec))
            sec -= sizes[-1]
        sections.append(tuple(sizes))
    plan = Plan(quotas=quotas, sections=tuple(sections))

    total_chunks = plan.total_chunks
    slots = total_chunks * 128

    idx16 = np.zeros((c.cores, 128, slots // 16), np.int16)
    ids_f32 = np.empty((c.cores, 128, total_chunks), np.float32)

    order = np.lexsort((r_all, w_all, b_all, core_all))
    so_r, so_w, so_b, so_core, so_id = (
        r_all[order], w_all[order], b_all[order], core_all[order], id_all[order]
    )
    core_starts = np.searchsorted(so_core, np.arange(c.cores + 1))

    for ci in range(c.cores):
        lo, hi = core_starts[ci], core_starts[ci + 1]
        rr, ii = so_r[lo:hi], so_id[lo:hi]
        rel = np.zeros(slots, np.int64)      # window-relative gather rows
        ids = np.full(slots, -1.0, np.float32)
        # slot offset of window section w
        sec_off = np.cumsum([0] + [c.nt * q * 128 for q in quotas])
        pos = 0
        # sorted order within a core is (b, w, r); cells land at
        # sec_off[w] + pos_in_window(b) * quotas[w] * 128 (last window's
        # cells rotated to match the rotated hook/processing order)
        rotpos = _rotpos(c)
        for b in range(c.nt):
            for w in range(c.nwin):
                cnt = counts[ci, b, w]
                if cnt:
                    p = rotpos[b] if w == c.nwin - 1 else b
                    off = sec_off[w] + p * quotas[w] * 128
                    rel[off : off + cnt] = rr[pos : pos + cnt] - wbases[w]
                    ids[off : off + cnt] = ii[pos : pos + cnt]
                    pos += cnt
        assert pos == hi - lo
        assert rel.min() >= 0
        for w in range(c.nwin):
            seg = rel[sec_off[w] : sec_off[w + 1]]
            assert seg.max(initial=0) < c.wsize

        v = rel.reshape(-1, 16)              # slot i at [i%16, i//16]
        wrapped = np.ascontiguousarray(v.T)  # [16, slots/16]
        idx16[ci] = np.tile(wrapped, (8, 1)).astype(np.int16)
        ids_f32[ci] = ids.reshape(total_chunks, 128).T

    return dinv_pt, sqdeg_r, idx16, ids_f32, plan


# ----------------------------------------------------------------------------
# Device kernel
# ----------------------------------------------------------------------------

def build(nc, tc, cfg: Cfg, plan: Plan):
    c = cfg
    RG = [list(range(c.cores))]
    total_chunks = plan.total_chunks
    slots = total_chunks * 128
    nsec = c.nsec
    stb = c.stb                             # table blocks per section

    x_t = nc.dram_tensor("x_t", [c.din, c.nloc], BF16, kind="ExternalInput").ap()
    w1 = nc.dram_tensor("w1", [c.din, c.dh], BF16, kind="ExternalInput").ap()
    w2 = nc.dram_tensor("w2", [c.dh, c.dt2], F32, kind="ExternalInput").ap()
    b1r = nc.dram_tensor("b1r", [1, c.dh], F32, kind="ExternalInput").ap()
    b2r = nc.dram_tensor("b2r", [1, c.dt2], F32, kind="ExternalInput").ap()
    dinvp = nc.dram_tensor("dinvp", [128, c.nt], F32, kind="ExternalInput").ap()
    sqdegr = nc.dram_tensor("sqdegr", [1, c.nloc], F32, kind="ExternalInput").ap()
    idx16 = nc.dram_tensor("idx16", [128, slots // 16], I16, kind="ExternalInput").ap()
    idsf = nc.dram_tensor("idsf", [128, total_chunks], BF16, kind="ExternalInput").ap()
    out_h = nc.dram_tensor("out_h", [c.nloc, c.dh2], F32, kind="ExternalOutput").ap()
    out_ls = nc.dram_tensor("out_ls", [c.nloc, c.dh2], F32, kind="ExternalOutput").ap()

    # local table shards (uniform sections) + full (shared) gather tables
    t1_loc = nc.dram_tensor("t1_loc", [c.tloc, c.dh], BF16, kind="Internal").ap()
    t1_full = nc.dram_tensor(
        "t1_full", [c.trows, c.dh], BF16, kind="Internal", addr_space="Shared"
    ).ap()
    t2_loc = nc.dram_tensor("t2_loc", [c.tloc, c.dt2], BF16, kind="Internal").ap()
    t2_full = nc.dram_tensor(
        "t2_full", [c.trows, c.dt2], BF16, kind="Internal", addr_space="Shared"
    ).ap()

    with ExitStack() as st:
        cpool = st.enter_context(tc.tile_pool(name="consts", bufs=1))
        accp = st.enter_context(tc.tile_pool(name="acc", bufs=1))
        gp = st.enter_context(tc.tile_pool(name="gp", bufs=c.gbufs))
        pp = st.enter_context(tc.tile_pool(name="pp", bufs=6))
        ppsum = st.enter_context(tc.tile_pool(name="ppsum", bufs=5, space="PSUM"))
        p0 = st.enter_context(tc.tile_pool(name="p0", bufs=3))
        p0ps = st.enter_context(tc.tile_pool(name="p0ps", bufs=2, space="PSUM"))
        p0psT = st.enter_context(tc.tile_pool(name="p0psT", bufs=1, space="PSUM"))

        # ---- constants ----
        ident = cpool.tile([128, 128], F32)
        make_identity(nc, ident)
        identb = cpool.tile([128, 128], BF16)
        make_identity(nc, identb)
        w1sb = cpool.tile([128, c.kt, c.dh], BF16)
        nc.sync.dma_start(w1sb, w1.rearrange("(o p) f -> p o f", p=128))
        w2sb = cpool.tile([128, c.dt2], F32)
        nc.sync.dma_start(w2sb, w2)
        b1sb = cpool.tile([1, c.dh], F32)
        nc.sync.dma_start(b1sb, b1r)
        b2sb = cpool.tile([1, c.dt2], F32)
        nc.sync.dma_start(b2sb, b2r)
        dinv = cpool.tile([128, c.nt], F32)
        nc.sync.dma_start(dinv, dinvp)
        iota = cpool.tile([128, c.max_piece, 128], BF16)
        nc.gpsimd.iota(iota, pattern=[[0, c.max_piece], [1, 128]], base=0,
                       channel_multiplier=0,
                       allow_small_or_imprecise_dtypes=True)

        # ---- whole-run gather indices + one-hot ids, loaded once ----
        sit = cpool.tile([128, total_chunks * 8], I16)
        nc.sync.dma_start(sit, idx16)
        sid = cpool.tile([128, total_chunks], BF16)
        nc.sync.dma_start(sid, idsf)

        # ---- phase 0: T1 = dinv * (x @ W1) ----
        xtv = x_t.rearrange("(k p) n -> p k n", p=128)   # [128, kt, nloc]

        def phase0_block(t):
            xTt = p0.tile([128, c.kt, 128], BF16, tag="xTt")
            nc.sync.dma_start(xTt, xtv[:, :, ts(t, 128)])
            hps = p0ps.tile([128, c.dh], F32, tag="hps")
            for j in range(c.kt):
                nc.tensor.matmul(
                    hps, lhsT=xTt[:, j, :], rhs=w1sb[:, j, :],
                    start=(j == 0), stop=(j == c.kt - 1),
                )
            hsb = p0.tile([128, c.dh], BF16, tag="hsb")
            nc.scalar.activation(hsb, hps, AF.Copy, scale=dinv[:, t : t + 1])
            nc.sync.dma_start(t1_loc[ts(t, 128), :], hsb)

        # Sub-AllGather of one section s: local rows [s*sr, (s+1)*sr) of
        # every core concatenate into gather window s.
        def emit_ag1(s):
            nc.gpsimd.collective_compute(
                "AllGather", ALU.bypass, replica_groups=RG,
                ins=[t1_loc[ds(s * c.sec_rows, c.sec_rows), :].opt()],
                outs=[t1_full[ds(c.wbases[s], c.wsize), :].opt()],
            )

        for t in range(c.nt):
            phase0_block(t)
            if t == stb - 1:     # section 0 complete -> unblock window 0
                emit_ag1(0)

        # ---- edge aggregation ----
        qctr = [0]
        LOOK = 3                # gather/stt emission lookahead (pieces)
        HLAG = 5                # hook emission lag (pieces)

        def edge_phase(table_full, t_loc, acc, d, dt, brow, hook, inject, worder):
            """acc[:, b, :d] = sum_edges T[src] + T[self] + sqdeg*b  per block.

            Software-pipelined emission: gather+one-hot of piece k+LOOK are
            emitted before the matmuls of piece k (so DVE one-hot builds
            never queue behind PSUM drains), and hook bodies for blocks
            completed in piece k are emitted after piece k+1's matmuls (so
            their PE/scalar round-trips never stall the piece stream).
            inject: {gather_piece_index: fn} slots collectives into the
            Pool instruction stream between gathers.
            """
            # precompute piece descriptors, windows in processing order
            wchunk0, acc_ch = [], 0
            for sizes in plan.sections:
                wchunk0.append(acc_ch)
                acc_ch += sum(sizes)
            pieces = []          # (w, nch, loc)
            for w in worder:
                loc = 0
                for nch in plan.sections[w]:
                    pieces.append((w, nch, loc))
                    loc += nch
            npc = len(pieces)
            wfirst, wlast = worder[0], worder[-1]
            rotseq = _rotseq(c)

            def blk_at(w, pos):
                return rotseq[pos] if w == c.nwin - 1 else pos
            gtile = {}
            sttile = {}

            def emit_fetch(k):
                w, nch, loc = pieces[k]
                if k in inject:
                    inject[k]()
                ch0 = wchunk0[w] + loc
                g = gp.tile([128, c.max_piece, dt], BF16, tag="gt")
                nc.gpsimd.dma_gather(
                    g[:, :nch, :], table_full[ds(c.wbases[w], c.wsize), :],
                    sit[:, ch0 * 8 : (ch0 + nch) * 8],
                    num_idxs=nch * 128, num_idxs_reg=nch * 128, elem_size=dt,
                    single_packet=False, queue_num=qctr[0] % 4,
                )
                qctr[0] += 1
                stt = pp.tile([128, c.max_piece, 128], BF16, tag="stt")
                nc.vector.tensor_tensor(
                    stt[:, :nch, :], iota[:, :nch, :],
                    sid[:, wchunk0[w] + loc : wchunk0[w] + loc + nch, None]
                    .to_broadcast((128, nch, 128)),
                    ALU.is_equal,
                )
                gtile[k] = g
                sttile[k] = stt

            tv = t_loc.rearrange("(b p) f -> p b f", p=128)

            def seed(b):
                sd = p0.tile([128, dt], BF16, tag="sd")
                nc.sync.dma_start(sd, tv[:, b, :])
                sq = p0.tile([1, 128], F32, tag="sq")
                nc.sync.dma_start(sq, sqdegr[:, ts(b, 128)])
                return sd, sq

            state = {"b": 0, "k_in_block": 0, "ps": None}

            def emit_consume(k, newhooks):
                w, nch, loc = pieces[k]
                q = plan.quotas[w]
                first, last = w == wfirst, w == wlast
                if loc == 0:
                    state["b"] = 0
                    state["k_in_block"] = 0
                g, stt = gtile.pop(k), sttile.pop(k)
                for j in range(nch):
                    b = blk_at(w, state["b"])
                    if state["k_in_block"] == 0:
                        ps_new = ppsum.tile([128, d], F32, tag="ps")
                        state["ps"] = ps_new
                        if first:
                            # chain seeds: self-loop row + rank-1 bias
                            sd, sq = seed(b)
                            nc.tensor.matmul(
                                state["ps"], lhsT=identb, rhs=sd[:, :d],
                                start=True, stop=False,
                            )
                            nc.tensor.matmul(
                                state["ps"], lhsT=sq, rhs=brow[:, :d],
                                start=False, stop=False,
                            )
                    nc.tensor.matmul(
                        state["ps"], lhsT=stt[:, j, :], rhs=g[:, j, :d],
                        start=(state["k_in_block"] == 0 and not first),
                        stop=(state["k_in_block"] == q - 1),
                    )
                    state["k_in_block"] += 1
                    if state["k_in_block"] == q:
                        ps = state["ps"]
                        if first:
                            nc.vector.tensor_copy(acc[:, b, :], ps)
                        else:
                            nc.vector.tensor_tensor(
                                acc[:, b, :], acc[:, b, :], ps, ALU.add
                            )
                            if last:
                                newhooks.append(b)
                        state["b"] += 1
                        state["k_in_block"] = 0

            for k in range(LOOK):
                emit_fetch(k)
            pending = []         # (piece, [blocks]) hook backlog
            for k in range(npc):
                if k + LOOK < npc:
                    emit_fetch(k + LOOK)
                newhooks = []
                emit_consume(k, newhooks)
                if newhooks:
                    pending.append((k, newhooks))
                while pending and pending[0][0] <= k - HLAG:
                    for b in pending.pop(0)[1]:
                        hook(b)
            for _, hs in pending:
                for b in hs:
                    hook(b)
            assert state["b"] == c.nt and state["k_in_block"] == 0

        # ---- layer 1, with per-block t2 construction + sub-AllGathers ----
        acc1 = accp.tile([128, c.nt, c.dh], BF16)

        def hook1(b):
            # g1 = dinv * relu(dinv * agg + b1); t2 rows = g1 @ W2
            zb = acc1[:, b, :]
            zr = p0.tile([128, c.dh], F32, tag="zr")
            nc.scalar.activation(zr, zb, AF.Relu, scale=dinv[:, b : b + 1])
            zs = p0.tile([128, c.dh], F32, tag="zs")
            nc.scalar.activation(zs, zr, AF.Copy, scale=dinv[:, b : b + 1])
            tps = p0psT.tile([128, 128], F32, tag="tps2")
            nc.tensor.transpose(tps, zs, ident)
            gT = p0.tile([128, 128], F32, tag="gT")
            nc.scalar.activation(gT, tps, AF.Copy)
            h2ps = p0ps.tile([128, c.dt2], F32, tag="hps")
            nc.tensor.matmul(h2ps, lhsT=gT, rhs=w2sb, start=True, stop=True)
            h2sb = p0.tile([128, c.dt2], BF16, tag="h2sb")
            nc.scalar.activation(h2sb, h2ps, AF.Copy)
            nc.sync.dma_start(t2_loc[ts(b, 128), :], h2sb)
            # hooks run in rotated order: each table section finishes
            # contiguously, and its sub-AllGather fires immediately
            s = b // stb
            if b == min((s + 1) * stb, c.nt) - 1:
                nc.gpsimd.collective_compute(
                    "AllGather", ALU.bypass, replica_groups=RG,
                    ins=[t2_loc[ds(s * c.sec_rows, c.sec_rows), :].opt()],
                    outs=[t2_full[ds(c.wbases[s], c.wsize), :].opt()],
                )

        edge_phase(
            t1_full, t1_loc, acc1, c.dh, c.dh, b1sb, hook1,
            inject={10: lambda: emit_ag1(1),
                    15: lambda: emit_ag1(2),
                    20: lambda: emit_ag1(3)},
            worder=(0, 1, 2, 3),
        )



        # ---- layer 2, per-block scale/max/exp; all DMAs batched at end ----
        acc2 = accp.tile([128, c.nt, c.dh2], F32)
        se_all = cpool.tile([128, c.nt], F32)
        znall = cpool.tile([128, c.nt, c.dout], F32)
        ohv = out_h.rearrange("(t p) f -> p t f", p=128)
        olv = out_ls.rearrange("(t p) f -> p t f", p=128)

        def hook2(b):
            # in place: acc2_b <- dinv * acc2_b  (the final h rows)
            nc.scalar.activation(acc2[:, b, :], acc2[:, b, :], AF.Copy,
                                 scale=dinv[:, b : b + 1])
            mx = p0.tile([128, 1], F32, tag="mx")
            nc.vector.tensor_reduce(mx, acc2[:, b, : c.dout],
                                    mybir.AxisListType.X, ALU.max)
            nc.vector.tensor_tensor(
                znall[:, b, :], acc2[:, b, : c.dout],
                mx.to_broadcast((128, c.dout)), ALU.subtract,
            )
            e1 = p0.tile([128, c.dout], F32, tag="e1")
            nc.scalar.activation(e1, znall[:, b, :], AF.Exp)
            nc.vector.tensor_reduce(se_all[:, b : b + 1], e1,
                                    mybir.AxisListType.X, ALU.add)

        edge_phase(t2_full, t2_loc, acc2, c.dh2, c.dt2, b2sb, hook2,
                   inject={}, worder=(3, 0, 1, 2))

        # batched tail: h out, then out_ls = (z - mx) - ln(sum exp)
        nc.sync.dma_start(ohv, acc2)
        lnall = cpool.tile([128, c.nt], F32)
        nc.scalar.activation(lnall, se_all, AF.Ln)
        nc.vector.tensor_tensor(
            znall, znall,
            lnall[:, :, None].to_broadcast((128, c.nt, c.dout)), ALU.subtract,
        )
        nc.sync.dma_start(olv[:, :, : c.dout], znall)


# ----------------------------------------------------------------------------
# Host entry point
# ----------------------------------------------------------------------------

_CACHE = {}


def _get_compiled(cfg: Cfg, plan: Plan):
    key = (cfg, plan)
    if key not in _CACHE:
        nc = bacc.Bacc(
            "TRN2", target_bir_lowering=False, debug=False,
            num_devices=cfg.cores, num_swdge_queues=4,
            dynamic_dma_scratch_size=32768,
        )
        with tile.TileContext(nc) as tc:
            build(nc, tc, cfg, plan)
        nc.compile()
        _CACHE[key] = nc
    return _CACHE[key]


def make_in_maps(cfg: Cfg, x, W1, b1, W2, b2, dinv_pt, sqdeg_r, idx16, ids_f32):
    import ml_dtypes

    c = cfg
    x = np.asarray(x, np.float32)
    w2p = np.zeros((c.dh, c.dt2), np.float32)
    w2p[:, : c.dout] = np.asarray(W2, np.float32)
    b1rep = np.asarray(b1, np.float32)[None, :]
    b2p = np.zeros((1, c.dt2), np.float32)
    b2p[0, : c.dout] = np.asarray(b2, np.float32)
    w1c = np.ascontiguousarray(
        np.asarray(W1, np.float32).astype(ml_dtypes.bfloat16)
    )

    in_maps = []
    for ci in range(c.cores):
        xs = np.zeros((c.din, c.nloc), ml_dtypes.bfloat16)
        xs[:, : c.nsh] = np.ascontiguousarray(
            x[ci * c.nsh : (ci + 1) * c.nsh].T
        ).astype(ml_dtypes.bfloat16)
        in_maps.append({
            "x_t": xs,
            "w1": w1c,
            "w2": w2p,
            "b1r": b1rep,
            "b2r": b2p,
            "dinvp": np.ascontiguousarray(dinv_pt[ci]),
            "sqdegr": np.ascontiguousarray(sqdeg_r[ci]),
            "idx16": np.ascontiguousarray(idx16[ci]),
            "idsf": np.ascontiguousarray(ids_f32[ci].astype(ml_dtypes.bfloat16)),
        })
    return in_maps


def _ensure_ntff_hook():
    """Install the axon NTFF profile hook if the image's antenv lacks it."""
    import types

    try:
        from antenv.axon_hooks import get_axon_ntff_profile_hook  # noqa: F401
        return
    except ImportError:
        pass
    import antenv

    m = types.ModuleType("antenv.axon_hooks")
    m._hook = None
    m.set_axon_ntff_profile_hook = lambda h: setattr(m, "_hook", h)
    m.get_axon_ntff_profile_hook = lambda: m._hook
    sys.modules["antenv.axon_hooks"] = m
    antenv.axon_hooks = m
    try:
        from trn_agent_boot.trn_boot import _ntff_profile_via_ctypes

        h = _ntff_profile_via_ctypes("/opt/axon/libaxon_pjrt.so")
        if h is not None:
            m._hook = h
    except Exception as e:
        print(f"ntff hook install failed: {e}")

    from concourse import bass_utils as bu

    bu.upload_artifacts = lambda tmpdir: tmpdir


def run(cfg: Cfg, inputs: dict, trace: bool = False):
    if trace:
        _ensure_ntff_hook()
    dinv_pt, sqdeg_r, idx16, ids_f32, plan = preprocess(cfg, inputs["edge_index"])
    nc = _get_compiled(cfg, plan)
    in_maps = make_in_maps(
        cfg, inputs["x"], inputs["W1"], inputs["b1"], inputs["W2"], inputs["b2"],
        dinv_pt, sqdeg_r, idx16, ids_f32,
    )
    res = run_bass_kernel_spmd(
        nc, in_maps, core_ids=list(range(cfg.cores)), trace=trace
    )
    c = cfg
    h = np.concatenate(
        [res.results[ci]["out_h"][: c.nsh, : c.dout] for ci in range(c.cores)], axis=0
    )
    ls = np.concatenate(
        [res.results[ci]["out_ls"][: c.nsh, : c.dout] for ci in range(c.cores)], axis=0
    )
    return (h, ls), res


def kernel(**inputs):
    (h, ls), _ = run(Cfg(), inputs)
    return h, ls
